# revision 4
# baseline (speedup 1.0000x reference)
"""Trainium2 Bass kernel v2 for DirectedNetworkFeatureExtractor (GAT+FC GNN).

Changes vs baseline:
- S0 (scatter one-hot) and S0T (ald-gather one-hot) are host-precomputed bf16
  DRAM inputs streamed per section (kills the big DVE is_equal passes + srR).
- AllGather output table is addr_space="Shared" (fast collective path) and AG
  ops are interleaved into the node phase as h windows complete.
- Aggregation accumulates across chunk-sections in PSUM (one evict per window).
- pe is exp-expanded on the Scalar engine into [P,nt,128] so the GW multiply
  runs in DVE 2x mode; als uses fold-then-reduce.
- Node phase and x1 phase share one load of the input state; L2 residual add
  is fused into the loads (no materialization pass).
"""
import math
import sys

sys.path.insert(0, "/opt/trn_rl_repo")

import numpy as np
import ml_dtypes

import concourse.bass as bass
import concourse.bacc as bacc
import concourse.tile as tile
from concourse import mybir

BF = ml_dtypes.bfloat16
P = 128

N_NODES = 100_000
N_CORES = 8
HEADS = 4
NCHUNK = 4
GB = 4          # windows per gather group


# --------------------------------------------------------------------------
# host-side graph preprocessing (untimed)
# --------------------------------------------------------------------------
def prep_structure(edge_index, n_nodes, n_cores):
    src = np.asarray(edge_index[0]).astype(np.int64)
    dst = np.asarray(edge_index[1]).astype(np.int64)
    shard = n_nodes // n_cores
    W = math.ceil(shard / P)
    SH = W * P
    TAB = n_cores * SH
    CHSZ = TAB // NCHUNK
    NE = len(src)

    core = dst // shard
    dloc = dst - core * shard
    win = dloc // P
    SHC = SH // NCHUNK                                # shard rows per AG chunk
    loc = src % shard
    rnk = src // shard
    chk = loc // SHC
    rel = (rnk * SHC + (loc - chk * SHC)).astype(np.int64)  # row in chunk table

    g_of_w = np.arange(W) // GB
    NG = math.ceil(W / GB)
    grp = g_of_w[win]
    cnt = np.zeros((n_cores, NG, NCHUNK), np.int64)
    np.add.at(cnt, (core, grp, chk), 1)
    Tgc = (cnt.max(axis=0) + P - 1) // P          # [NG, NCHUNK] tiles per cell

    # per-core sort: (core, group, chunk, dloc)
    order = np.lexsort((np.arange(NE), dloc, chk, grp, core))
    s_core, s_grp, s_chk = core[order], grp[order], chk[order]
    s_dloc, s_rel, s_win = dloc[order], rel[order], win[order]
    gid = (s_core * NG + s_grp) * NCHUNK + s_chk
    first = np.r_[True, gid[1:] != gid[:-1]]
    starts = np.flatnonzero(first)
    run_id = np.cumsum(first) - 1
    pos = np.arange(NE) - starts[run_id]

    # global tile base per (g, c)
    base = np.zeros((NG, NCHUNK), np.int64)
    t_run = 0
    for g in range(NG):
        for c in range(NCHUNK):
            base[g, c] = t_run
            t_run += int(Tgc[g, c])
    TT = t_run
    tile_i = base[s_grp, s_chk] + pos // P
    part_i = pos % P

    # (tile, window) pair union across cores
    pair_set = set(zip(tile_i.tolist(), s_win.tolist()))
    # every tile needs >=1 pair; padded tiles may have none -> give them (t, first window of group)
    tile_g = np.zeros(TT, np.int64)
    for g in range(NG):
        for c in range(NCHUNK):
            tile_g[base[g, c]:base[g, c] + Tgc[g, c]] = g
    for t in range(TT):
        if not any((t, w) in pair_set for w in range(tile_g[t] * GB, min((tile_g[t] + 1) * GB, W))):
            pair_set.add((t, int(tile_g[t] * GB)))
    pairs = sorted(pair_set)                       # ordered by (tile, win)
    pair_col = {pw: i for i, pw in enumerate(pairs)}
    PB = len(pairs)

    # plan: per group/section: ntiles, pair lists
    plan = []
    for g in range(NG):
        ws = list(range(g * GB, min((g + 1) * GB, W)))
        g_off = int(base[g, 0])
        secs = []
        for c in range(NCHUNK):
            s_off = int(base[g, c])
            nt = int(Tgc[g, c])
            sec_pairs = [(t, w) for (t, w) in pairs
                         if s_off <= t < s_off + nt]
            pbase = pair_col[sec_pairs[0]] if sec_pairs else 0
            # per-tile pair list (local pair col), per-window run list
            tile_pairs = [[] for _ in range(nt)]
            win_runs = {}
            for (t, w) in sec_pairs:
                lp = pair_col[(t, w)] - pbase
                tile_pairs[t - s_off].append((w, lp))
                win_runs.setdefault(w, []).append((t - s_off, lp))
            secs.append(dict(off=s_off, ntiles=nt, pbase=pbase,
                             npairs=len(sec_pairs), tile_pairs=tile_pairs,
                             win_runs=sorted(win_runs.items())))
        gnt = sum(s["ntiles"] for s in secs)
        plan.append(dict(windows=ws, off=g_off, ntiles=gnt, secs=secs))

    idx16 = np.zeros((n_cores, TT * P), np.int16)
    idx16[s_core, tile_i * P + part_i] = s_rel.astype(np.int16)

    # host-built one-hots over pair columns
    ONE = np.uint16(0x3F80)
    s0 = np.zeros((n_cores, P, PB * P), np.uint16)
    s0t = np.zeros((n_cores, P, PB * P), np.uint16)
    pcol_of_edge = np.array([pair_col[(t, w)] for t, w in zip(tile_i.tolist(), s_win.tolist())],
                            dtype=np.int64)
    slot = (s_dloc - s_win * P).astype(np.int64)
    s0[s_core, part_i, pcol_of_edge * P + slot] = ONE
    s0t[s_core, slot, pcol_of_edge * P + part_i] = ONE
    # pack idx: element i at [r, i//16] for all r with r%16 == i%16
    j = np.arange(TT * 8)
    r16 = np.arange(16)
    packed = idx16[:, (j[None, :] * 16 + r16[:, None]).reshape(16, -1)]
    idx_packed = np.tile(packed, (1, 8, 1))       # [cores, 128, TT*8]

    return dict(
        shard=shard, W=W, SH=SH, TT=TT, TAB=TAB, CHSZ=CHSZ, NG=NG, plan=plan,
        PB=PB,
        idx=np.ascontiguousarray(idx_packed),
        s0=s0.view(BF),
        s0t=s0t.view(BF),
    )


def prep_weights(inputs):
    def blocks(w):
        k = w.shape[0]
        return np.ascontiguousarray(w.reshape(k // P, P, w.shape[1]).astype(BF))

    def rep_row(v):
        return np.broadcast_to(np.asarray(v, np.float32), (P, P)).copy()

    g = lambda n: np.asarray(inputs[n], np.float32)
    layers = [dict(
        gw=blocks(g("g1_W")), fw=blocks(g("fc1_W")),
        a_s=rep_row(g("g1_as").reshape(-1)).astype(BF),
        a_d=rep_row(g("g1_ad").reshape(-1)).astype(BF),
        gb=rep_row(g("g1_b")), fb=g("fc1_b").reshape(P, 1).astype(np.float32),
    )]
    for i in range(2):
        layers.append(dict(
            gw=blocks(g("mg_W")[i]), fw=blocks(g("mfc_W")[i]),
            a_s=rep_row(g("mg_as")[i].reshape(-1)).astype(BF),
            a_d=rep_row(g("mg_ad")[i].reshape(-1)).astype(BF),
            gb=rep_row(g("mg_b")[i]), fb=g("mfc_b")[i].reshape(P, 1).astype(np.float32),
        ))
    layers.append(dict(
        gw=blocks(g("fg_W")), fw=blocks(g("ffc_W")),
        a_s=rep_row(g("fg_as").reshape(-1)).astype(BF),
        a_d=rep_row(g("fg_ad").reshape(-1)).astype(BF),
        gb=rep_row(g("fg_b")), fb=rep_row(g("ffc_b")),
    ))
    return layers


# --------------------------------------------------------------------------
# device program
# --------------------------------------------------------------------------
def build_program(st, n_cores):
    SH, W, TT, TAB, CHSZ = st["SH"], st["W"], st["TT"], st["TAB"], st["CHSZ"]
    PB = st["PB"]
    SHC = SH // NCHUNK
    plan = st["plan"]
    dt = mybir.dt
    f32, bf16, i16 = dt.float32, dt.bfloat16, dt.int16
    HL = [HEADS, HEADS, HEADS, 1]
    GNTMAX = max(g["ntiles"] for g in plan)
    SECMAX = max(s["ntiles"] for g in plan for s in g["secs"])
    SECPMAX = max(s["npairs"] for g in plan for s in g["secs"])

    nc = bacc.Bacc(None)

    def inp(name, shape, d):
        return nc.declare_dram_parameter(name, list(shape), d, isOutput=False)

    x_in = inp("x", (SH, P), f32)
    idx_in = inp("idx", (P, TT * 8), i16)
    s0_in = inp("s0", (P, PB * P), bf16)
    s0t_in = inp("s0t", (P, PB * P), bf16)
    iota_in = inp("iota8", (P, P), dt.int8)
    iotac_in = inp("iotac8", (P, 1), dt.int8)
    lw = []
    for L in range(4):
        K = 1 if L == 0 else 2
        lw.append(dict(
            gw=inp(f"gw{L}", (K, P, P), bf16),
            fw=inp(f"fw{L}", (K, P, P), bf16),
            a_s=inp(f"as{L}", (P, P), bf16),
            a_d=inp(f"ad{L}", (P, P), bf16),
            gb=inp(f"gb{L}", (P, P), f32),
            fb=inp(f"fb{L}", (P, 1) if L < 3 else (P, P), f32),
        ))
    out_t = nc.declare_dram_parameter("out", [SH, P], f32, isOutput=True)

    CH = 512
    chunks = [(c, min(CH, SH - c)) for c in range(0, SH, CH)]

    with tile.TileContext(nc) as tc:
        with (
            tc.tile_pool(name="res", bufs=1) as res,
            tc.tile_pool(name="wts", bufs=1) as wts,
            tc.tile_pool(name="nwork", bufs=3) as nwork,
            tc.tile_pool(name="ework", bufs=2) as ework,
            tc.tile_pool(name="psA", bufs=2, space="PSUM") as psA,
            tc.tile_pool(name="psB", bufs=2, space="PSUM") as psB,
            tc.tile_pool(name="dram", bufs=1, space="DRAM") as dram,
        ):
            # ---------------- residents
            iota8 = res.tile([P, P], dt.int8)
            nc.sync.dma_start(iota8[:], iota_in[:])
            iotac8 = res.tile([P, 1], dt.int8)
            nc.sync.dma_start(iotac8[:], iotac_in[:])
            ident = res.tile([P, P], bf16)
            nc.vector.tensor_tensor(
                out=ident[:], in0=iotac8[:].to_broadcast([P, P]), in1=iota8[:],
                op=mybir.AluOpType.is_equal)

            wt = []
            for L in range(4):
                K = 1 if L == 0 else 2
                d = {}
                for nm in ("gw", "fw"):
                    t_ = wts.tile([P, K, P], bf16, tag=f"{nm}{L}")
                    nc.sync.dma_start(t_[:], lw[L][nm][:].rearrange("k p q -> p k q"))
                    d[nm] = t_
                for nm in ("a_s", "a_d"):
                    t_ = wts.tile([P, P], bf16, tag=f"{nm}{L}")
                    nc.sync.dma_start(t_[:], lw[L][nm][:])
                    d[nm] = t_
                t_ = wts.tile([P, P], f32, tag=f"gb{L}")
                nc.sync.dma_start(t_[:], lw[L]["gb"][:])
                d["gb"] = t_
                t_ = wts.tile([P, 1] if L < 3 else [P, P], f32, tag=f"fb{L}")
                nc.sync.dma_start(t_[:], lw[L]["fb"][:])
                d["fb"] = t_
                wt.append(d)

            # ---------------- DRAM scratch
            sA = [dram.tile([P, SH], bf16, tag=f"sA{i}", name=f"sA{i}") for i in range(3)]
            sB = [dram.tile([P, SH], bf16, tag=f"sB{i}", name=f"sB{i}") for i in range(3)]
            h_bounce = dram.tile([SH, P], bf16, tag="hb")
            tables = [[dram.tile([CHSZ, P], bf16, tag=f"tab{L}_{c}",
                                 name=f"tab{L}_{c}", addr_space="Shared")
                       for c in range(NCHUNK)] for L in range(4)]
            x4_dram = dram.tile([SH, P], bf16, tag="x4")

            # ---------------- x -> transposed state
            for i in range(W):
                xt = nwork.tile([P, P], f32, tag="xin")
                nc.sync.dma_start(xt[:], x_in[i * P:(i + 1) * P, :])
                xb = nwork.tile([P, P], bf16, tag="xbf")
                nc.vector.tensor_copy(out=xb[:], in_=xt[:])
                tp = psB.tile([P, P], bf16, tag="tp", bufs=1)
                nc.tensor.transpose(out=tp[:], in_=xb[:], identity=ident[:])
                xTb = nwork.tile([P, P], bf16, tag="xT")
                nc.vector.tensor_copy(out=xTb[:], in_=tp[:])
                nc.sync.dma_start(sA[0][:, i * P:(i + 1) * P], xTb[:])

            # ---------------- layers
            for L in range(4):
                K = 1 if L == 0 else 2
                H = HL[L]
                C = P // H
                w = wt[L]
                resid = False
                if L == 0:
                    in_blk = [sA[0]]
                elif L == 1:
                    in_blk = [sA[1], sB[1]]
                elif L == 2:
                    resid = True
                    in_blk = [(sA[1], sA[2]), (sB[1], sB[2])]
                else:
                    in_blk = [sA[1], sB[1]]

                # ---- node phase (fused h + x1 + ald, AG interleaved)
                ald_sb = res.tile([P, W, 4], f32, tag=f"aldsb{L % 2}")
                if H < 4:
                    nc.vector.memset(ald_sb[:], 0.0)
                outA = [sA[1], sA[2], sA[1], None][L]
                next_ag = 0
                for c0, cl in chunks:
                    nwin = cl // P
                    ins = []
                    for k in range(K):
                        it = nwork.tile([P, CH], bf16, tag=f"in{k}")
                        if resid:
                            b1, b2 = in_blk[k]
                            nc.sync.dma_start(it[:, :cl], b1[:, c0:c0 + cl])
                            it2 = nwork.tile([P, CH], bf16, tag=f"in2{k}")
                            nc.sync.dma_start(it2[:, :cl], b2[:, c0:c0 + cl])
                            nc.vector.tensor_add(out=it[:, :cl], in0=it[:, :cl],
                                                 in1=it2[:, :cl])
                        else:
                            nc.sync.dma_start(it[:, :cl], in_blk[k][:, c0:c0 + cl])
                        ins.append(it)
                    # x1 branch
                    if L < 3:
                        acc = psB.tile([P, CH], f32, tag="x1p")
                        for k in range(K):
                            nc.tensor.matmul(out=acc[:, :cl], lhsT=w["fw"][:, k, :],
                                             rhs=ins[k][:, :cl],
                                             start=(k == 0), stop=(k == K - 1))
                        x1b = nwork.tile([P, CH], bf16, tag="x1b")
                        nc.scalar.activation(out=x1b[:, :cl], in_=acc[:, :cl],
                                             func=mybir.ActivationFunctionType.Relu,
                                             bias=w["fb"][:], scale=1.0)
                        nc.sync.dma_start(outA[:, c0:c0 + cl], x1b[:, :cl])
                    # gat h per window (+ x4 for L==3)
                    for j in range(nwin):
                        i = c0 // P + j
                        sl = slice(j * P, (j + 1) * P)
                        hp = psA.tile([P, P], f32, tag="hp")
                        for k in range(K):
                            nc.tensor.matmul(out=hp[:], lhsT=ins[k][:, sl],
                                             rhs=w["gw"][:, k, :],
                                             start=(k == 0), stop=(k == K - 1))
                        hb = nwork.tile([P, P], bf16, tag="hbf")
                        nc.vector.tensor_copy(out=hb[:], in_=hp[:])
                        nc.sync.dma_start(h_bounce[i * P:(i + 1) * P, :], hb[:])
                        tm = nwork.tile([P, P], bf16, tag="adtmp")
                        nc.vector.tensor_tensor(out=tm[:], in0=hb[:], in1=w["a_d"][:],
                                                op=mybir.AluOpType.mult)
                        nc.vector.reduce_sum(
                            out=ald_sb[:, i, 0:H],
                            in_=tm[:].rearrange("p (h c) -> p h c", h=H),
                            axis=mybir.AxisListType.X)
                        if L == 3:
                            xp = psB.tile([P, CH], f32, tag="x1p")
                            for k in range(K):
                                nc.tensor.matmul(out=xp[:, :P], lhsT=ins[k][:, sl],
                                                 rhs=w["fw"][:, k, :],
                                                 start=(k == 0), stop=(k == K - 1))
                            x4t = nwork.tile([P, P], f32, tag="x4t")
                            nc.vector.tensor_add(out=x4t[:], in0=xp[:, :P], in1=w["fb"][:])
                            nc.vector.tensor_scalar_max(out=x4t[:], in0=x4t[:], scalar1=0.0)
                            x4b = nwork.tile([P, P], bf16, tag="x4b")
                            nc.vector.tensor_copy(out=x4b[:], in_=x4t[:])
                            nc.sync.dma_start(x4_dram[i * P:(i + 1) * P, :], x4b[:])
                        # fire chunk AG ops whose input rows are now complete
                        while next_ag < NCHUNK and (next_ag + 1) * SHC <= (i + 1) * P:
                            a0 = next_ag * SHC
                            nc.gpsimd.collective_compute(
                                "AllGather", mybir.AluOpType.bypass,
                                replica_groups=[list(range(n_cores))],
                                ins=[h_bounce[a0:a0 + SHC, :]],
                                outs=[tables[L][next_ag][:]],
                            )
                            next_ag += 1
                ald_bf = res.tile([P, W * 4], bf16, tag=f"aldbf{L % 2}")
                nc.vector.tensor_copy(out=ald_bf[:], in_=ald_sb[:].rearrange("p w c -> p (w c)"))

                # ---- edge phase
                for g in plan:
                    gt0, gnt = g["off"], g["ntiles"]
                    idxg = ework.tile([P, GNTMAX * 8], i16, tag="idxg", bufs=3)
                    nc.sync.dma_start(idxg[:, :gnt * 8], idx_in[:, gt0 * 8:(gt0 + gnt) * 8])
                    G = ework.tile([P, GNTMAX, P], bf16, tag="G", bufs=3)
                    for c in range(NCHUNK):
                        sec = g["secs"][c]
                        nt = sec["ntiles"]
                        if nt == 0:
                            continue
                        sl = sec["off"] - gt0
                        nc.gpsimd.dma_gather(
                            G[:, sl:sl + nt, :],
                            tables[L][c][:],
                            idxg[:, sl * 8:(sl + nt) * 8],
                            num_idxs=nt * P, num_idxs_reg=nt * P, elem_size=P,
                            single_packet=False)

                    als = ework.tile([P, GNTMAX * 4], f32, tag="als")
                    lg = ework.tile([P, GNTMAX * 4], f32, tag="lg")
                    for c in range(NCHUNK):
                        sec = g["secs"][c]
                        nt = sec["ntiles"]
                        if nt == 0:
                            continue
                        sl = sec["off"] - gt0
                        # --- attention logits: als (fold then reduce)
                        tmp = ework.tile([P, SECMAX, P], bf16, tag="tmp")
                        nc.vector.tensor_tensor(
                            out=tmp[:, :nt, :], in0=G[:, sl:sl + nt, :],
                            in1=w["a_s"][:].rearrange("p q -> p () q").to_broadcast([P, nt, P]),
                            op=mybir.AluOpType.mult)
                        tmpv = tmp[:, :nt, :].rearrange("p t (h c) -> p (t h) c", h=H)
                        nc.vector.tensor_add(
                            out=tmpv[:, :, 0:C // 2], in0=tmpv[:, :, 0:C // 2],
                            in1=tmpv[:, :, C // 2:C])
                        nc.vector.tensor_add(
                            out=tmpv[:, :, 0:C // 4], in0=tmpv[:, :, 0:C // 4],
                            in1=tmpv[:, :, C // 4:C // 2])
                        nc.vector.reduce_sum(
                            out=als[:, sl * H:(sl + nt) * H],
                            in_=tmpv[:, :, 0:C // 4],
                            axis=mybir.AxisListType.X)
                        # --- per-edge ald via host S0T one-hot matmul
                        npr = sec["npairs"]
                        pbase = sec["pbase"]
                        s0t_sec = ework.tile([P, SECPMAX * P], bf16, tag="s0t")
                        nc.sync.dma_start(
                            s0t_sec[:, :npr * P],
                            s0t_in[:, pbase * P:(pbase + npr) * P])
                        aldp = psA.tile([P, SECMAX, 4], f32, tag="ald", bufs=1)
                        for lt in range(nt):
                            prs = sec["tile_pairs"][lt]
                            for j, (wj, lp) in enumerate(prs):
                                nc.tensor.matmul(
                                    out=aldp[:, lt, 0:H],
                                    lhsT=s0t_sec[:, lp * P:(lp + 1) * P],
                                    rhs=ald_bf[:, wj * 4:wj * 4 + H],
                                    start=(j == 0), stop=(j == len(prs) - 1))
                        nc.vector.tensor_add(
                            out=lg[:, sl * H:(sl + nt) * H],
                            in0=als[:, sl * H:(sl + nt) * H],
                            in1=aldp[:, :nt, 0:H].rearrange("p t h -> p (t h)"))

                    lr = ework.tile([P, GNTMAX * 4], f32, tag="lr")
                    nc.vector.scalar_tensor_tensor(
                        out=lr[:, :gnt * H], in0=lg[:, :gnt * H], scalar=0.2,
                        in1=lg[:, :gnt * H],
                        op0=mybir.AluOpType.mult, op1=mybir.AluOpType.max)

                    x2acc = ework.tile([P, GB, P + 4], f32, tag="x2acc")
                    wdone = {}
                    for c in range(NCHUNK):
                        sec = g["secs"][c]
                        nt = sec["ntiles"]
                        if nt == 0:
                            continue
                        sl = sec["off"] - gt0
                        # pe expanded to [P, nt, H, C] on the scalar engine
                        pex = ework.tile([P, SECMAX, P], bf16, tag="pex")
                        nc.scalar.activation(
                            out=pex[:, :nt, :].rearrange("p t (h c) -> p t h c", h=H),
                            in_=lr[:, sl * H:(sl + nt) * H]
                                .rearrange("p (t h) -> p t h ()", h=H)
                                .to_broadcast([P, nt, H, C]),
                            func=mybir.ActivationFunctionType.Exp)
                        pe_t = ework.tile([P, SECMAX * 4], f32, tag="pe")
                        nc.scalar.activation(
                            out=pe_t[:, :nt * H], in_=lr[:, sl * H:(sl + nt) * H],
                            func=mybir.ActivationFunctionType.Exp)
                        GW = ework.tile([P, SECMAX, P + 4], bf16, tag="GW")
                        nc.vector.tensor_tensor(
                            out=GW[:, :nt, 0:P],
                            in0=G[:, sl:sl + nt, :],
                            in1=pex[:, :nt, :],
                            op=mybir.AluOpType.mult)
                        nc.vector.tensor_copy(
                            out=GW[:, :nt, P:P + H],
                            in_=pe_t[:, :nt * H].rearrange("p (t h) -> p t h", h=H))
                        s0_sec = ework.tile([P, SECPMAX * P], bf16, tag="s0")
                        nc.sync.dma_start(
                            s0_sec[:, :sec["npairs"] * P],
                            s0_in[:, sec["pbase"] * P:(sec["pbase"] + sec["npairs"]) * P])
                        for (wi, run) in sec["win_runs"]:
                            aggp = psA.tile([P, P + 4], f32, tag="agg")
                            for j, (lt, lp) in enumerate(run):
                                nc.tensor.matmul(
                                    out=aggp[:, :P + H],
                                    lhsT=s0_sec[:, lp * P:(lp + 1) * P],
                                    rhs=GW[:, lt, 0:P + H],
                                    start=(j == 0), stop=(j == len(run) - 1))
                            wl = wi - g["windows"][0]
                            if wi not in wdone:
                                wdone[wi] = True
                                nc.vector.tensor_copy(
                                    out=x2acc[:, wl, 0:P + H], in_=aggp[:, :P + H])
                            else:
                                nc.vector.tensor_add(
                                    out=x2acc[:, wl, 0:P + H],
                                    in0=x2acc[:, wl, 0:P + H], in1=aggp[:, :P + H])

                    for wi in g["windows"]:
                        wl = wi - g["windows"][0]
                        if wi not in wdone:
                            continue
                        sinv = ework.tile([P, 4], f32, tag="sinv")
                        nc.vector.tensor_scalar_add(
                            out=sinv[:, :H], in0=x2acc[:, wl, P:P + H], scalar1=1e-16)
                        nc.vector.reciprocal(out=sinv[:, :H], in_=sinv[:, :H])
                        x2 = ework.tile([P, P], f32, tag="x2")
                        nc.vector.tensor_tensor(
                            out=x2[:].rearrange("p (h c) -> p h c", h=H),
                            in0=x2acc[:, wl, 0:P].rearrange("p (h c) -> p h c", h=H),
                            in1=sinv[:, :H].rearrange("p h -> p h ()").to_broadcast([P, H, C]),
                            op=mybir.AluOpType.mult)
                        nc.vector.tensor_add(out=x2[:], in0=x2[:], in1=w["gb"][:])
                        nc.vector.tensor_scalar_max(out=x2[:], in0=x2[:], scalar1=0.0)
                        if L < 3:
                            x2b = ework.tile([P, P], bf16, tag="x2b")
                            nc.vector.tensor_copy(out=x2b[:], in_=x2[:])
                            tp = psB.tile([P, P], bf16, tag="tp", bufs=1)
                            nc.tensor.transpose(out=tp[:], in_=x2b[:], identity=ident[:])
                            x2T = ework.tile([P, P], bf16, tag="x2T")
                            nc.vector.tensor_copy(out=x2T[:], in_=tp[:])
                            outB = [sB[1], sB[2], sB[1]][L]
                            nc.sync.dma_start(outB[:, wi * P:(wi + 1) * P], x2T[:])
                        else:
                            x4t = ework.tile([P, P], bf16, tag="x4in")
                            nc.sync.dma_start(x4t[:], x4_dram[wi * P:(wi + 1) * P, :])
                            yo = ework.tile([P, P], f32, tag="yo")
                            nc.vector.tensor_add(out=yo[:], in0=x2[:], in1=x4t[:])
                            nc.sync.dma_start(out_t[wi * P:(wi + 1) * P, :], yo[:])

    nc.compile()
    return nc


# --------------------------------------------------------------------------
# runner
# --------------------------------------------------------------------------
def make_in_maps(inputs, st):
    x = np.asarray(inputs["x"], np.float32)
    shard, SH = st["shard"], st["SH"]
    layers = prep_weights(inputs)
    iota8 = np.broadcast_to(np.arange(P, dtype=np.int8), (P, P)).copy()
    iotac8 = np.arange(P, dtype=np.int8).reshape(P, 1).copy()

    common = {"iota8": iota8, "iotac8": iotac8}
    for L, lwd in enumerate(layers):
        common[f"gw{L}"] = lwd["gw"]
        common[f"fw{L}"] = lwd["fw"]
        common[f"as{L}"] = lwd["a_s"]
        common[f"ad{L}"] = lwd["a_d"]
        common[f"gb{L}"] = lwd["gb"]
        common[f"fb{L}"] = lwd["fb"]

    in_maps = []
    for c in range(N_CORES):
        xs = np.zeros((SH, P), np.float32)
        xs[:shard] = x[c * shard:(c + 1) * shard]
        m = dict(common)
        m["x"] = xs
        m["idx"] = st["idx"][c]
        m["s0"] = st["s0"][c]
        m["s0t"] = st["s0t"][c]
        in_maps.append(m)
    return in_maps


_CACHE = {}


def run(inputs, trace=False):
    from concourse.bass_utils import run_bass_kernel_spmd

    st = prep_structure(np.asarray(inputs["edge_index"]), N_NODES, N_CORES)
    key = (st["SH"], st["TT"])
    if key not in _CACHE:
        _CACHE[key] = build_program(st, N_CORES)
    nc = _CACHE[key]
    in_maps = make_in_maps(inputs, st)
    res = run_bass_kernel_spmd(nc, in_maps, core_ids=list(range(N_CORES)),
                               trace=trace)
    outs = [np.asarray(res.results[c]["out"])[:st["shard"]] for c in range(N_CORES)]
    return np.concatenate(outs, axis=0).astype(np.float32), res


def kernel(**inputs):
    out, _ = run(inputs, trace=False)
    return out


# revision 7
# speedup vs baseline: 1.1456x; 1.1456x over previous
"""Trainium2 Bass kernel v2 for DirectedNetworkFeatureExtractor (GAT+FC GNN).

Changes vs baseline:
- S0 (scatter one-hot) and S0T (ald-gather one-hot) are host-precomputed bf16
  DRAM inputs streamed per section (kills the big DVE is_equal passes + srR).
- AllGather output table is addr_space="Shared" (fast collective path) and AG
  ops are interleaved into the node phase as h windows complete.
- Aggregation accumulates across chunk-sections in PSUM (one evict per window).
- pe is exp-expanded on the Scalar engine into [P,nt,128] so the GW multiply
  runs in DVE 2x mode; als uses fold-then-reduce.
- Node phase and x1 phase share one load of the input state; L2 residual add
  is fused into the loads (no materialization pass).
"""
import math
import sys

sys.path.insert(0, "/opt/trn_rl_repo")

import numpy as np
import ml_dtypes

import concourse.bass as bass
import concourse.bacc as bacc
import concourse.tile as tile
from concourse import mybir

BF = ml_dtypes.bfloat16
P = 128

N_NODES = 100_000
N_CORES = 8
HEADS = 4
NCHUNK = 4
GB = 4          # windows per gather group


# --------------------------------------------------------------------------
# host-side graph preprocessing (untimed)
# --------------------------------------------------------------------------
def prep_structure(edge_index, n_nodes, n_cores):
    src = np.asarray(edge_index[0]).astype(np.int64)
    dst = np.asarray(edge_index[1]).astype(np.int64)
    shard = n_nodes // n_cores
    W = math.ceil(shard / P)
    SH = W * P
    TAB = n_cores * SH
    CHSZ = TAB // NCHUNK
    NE = len(src)

    core = dst // shard
    dloc = dst - core * shard
    win = dloc // P
    SHC = SH // NCHUNK                                # shard rows per AG chunk
    loc = src % shard
    rnk = src // shard
    chk = loc // SHC
    rel = (rnk * SHC + (loc - chk * SHC)).astype(np.int64)  # row in chunk table

    g_of_w = np.arange(W) // GB
    NG = math.ceil(W / GB)
    grp = g_of_w[win]
    cnt = np.zeros((n_cores, NG, NCHUNK), np.int64)
    np.add.at(cnt, (core, grp, chk), 1)
    Tgc = (cnt.max(axis=0) + P - 1) // P          # [NG, NCHUNK] tiles per cell

    # per-core sort: (core, group, chunk, dloc)
    order = np.lexsort((np.arange(NE), dloc, chk, grp, core))
    s_core, s_grp, s_chk = core[order], grp[order], chk[order]
    s_dloc, s_rel, s_win = dloc[order], rel[order], win[order]
    gid = (s_core * NG + s_grp) * NCHUNK + s_chk
    first = np.r_[True, gid[1:] != gid[:-1]]
    starts = np.flatnonzero(first)
    run_id = np.cumsum(first) - 1
    pos = np.arange(NE) - starts[run_id]

    # global tile base per (g, c)
    base = np.zeros((NG, NCHUNK), np.int64)
    t_run = 0
    for g in range(NG):
        for c in range(NCHUNK):
            base[g, c] = t_run
            t_run += int(Tgc[g, c])
    TT = t_run
    tile_i = base[s_grp, s_chk] + pos // P
    part_i = pos % P

    # (tile, window) pair union across cores
    pair_set = set(zip(tile_i.tolist(), s_win.tolist()))
    # every tile needs >=1 pair; padded tiles may have none -> give them (t, first window of group)
    tile_g = np.zeros(TT, np.int64)
    for g in range(NG):
        for c in range(NCHUNK):
            tile_g[base[g, c]:base[g, c] + Tgc[g, c]] = g
    for t in range(TT):
        if not any((t, w) in pair_set for w in range(tile_g[t] * GB, min((tile_g[t] + 1) * GB, W))):
            pair_set.add((t, int(tile_g[t] * GB)))
    pairs = sorted(pair_set)                       # ordered by (tile, win)
    pair_col = {pw: i for i, pw in enumerate(pairs)}
    PB = len(pairs)

    # plan: per group/section: ntiles, pair lists
    plan = []
    for g in range(NG):
        ws = list(range(g * GB, min((g + 1) * GB, W)))
        g_off = int(base[g, 0])
        secs = []
        for c in range(NCHUNK):
            s_off = int(base[g, c])
            nt = int(Tgc[g, c])
            sec_pairs = [(t, w) for (t, w) in pairs
                         if s_off <= t < s_off + nt]
            pbase = pair_col[sec_pairs[0]] if sec_pairs else 0
            # per-tile pair list (local pair col), per-window run list
            tile_pairs = [[] for _ in range(nt)]
            win_runs = {}
            for (t, w) in sec_pairs:
                lp = pair_col[(t, w)] - pbase
                tile_pairs[t - s_off].append((w, lp))
                win_runs.setdefault(w, []).append((t - s_off, lp))
            secs.append(dict(off=s_off, ntiles=nt, pbase=pbase,
                             npairs=len(sec_pairs), tile_pairs=tile_pairs,
                             win_runs=sorted(win_runs.items())))
        gnt = sum(s["ntiles"] for s in secs)
        plan.append(dict(windows=ws, off=g_off, ntiles=gnt, secs=secs))

    idx16 = np.zeros((n_cores, TT * P), np.int16)
    idx16[s_core, tile_i * P + part_i] = s_rel.astype(np.int16)

    # host-built one-hots over pair columns
    ONE = np.uint16(0x3F80)
    s0 = np.zeros((n_cores, P, PB * P), np.uint16)
    s0t = np.zeros((n_cores, P, PB * P), np.uint16)
    pcol_of_edge = np.array([pair_col[(t, w)] for t, w in zip(tile_i.tolist(), s_win.tolist())],
                            dtype=np.int64)
    slot = (s_dloc - s_win * P).astype(np.int64)
    s0[s_core, part_i, pcol_of_edge * P + slot] = ONE
    s0t[s_core, slot, pcol_of_edge * P + part_i] = ONE
    # pack idx: element i at [r, i//16] for all r with r%16 == i%16
    j = np.arange(TT * 8)
    r16 = np.arange(16)
    packed = idx16[:, (j[None, :] * 16 + r16[:, None]).reshape(16, -1)]
    idx_packed = np.tile(packed, (1, 8, 1))       # [cores, 128, TT*8]

    return dict(
        shard=shard, W=W, SH=SH, TT=TT, TAB=TAB, CHSZ=CHSZ, NG=NG, plan=plan,
        PB=PB,
        idx=np.ascontiguousarray(idx_packed),
        s0=s0.view(BF),
        s0t=s0t.view(BF),
    )


def prep_weights(inputs):
    def blocks(w):
        k = w.shape[0]
        return np.ascontiguousarray(w.reshape(k // P, P, w.shape[1]).astype(BF))

    def rep_row(v):
        return np.broadcast_to(np.asarray(v, np.float32), (P, P)).copy()

    g = lambda n: np.asarray(inputs[n], np.float32)
    layers = [dict(
        gw=blocks(g("g1_W")), fw=blocks(g("fc1_W")),
        a_s=rep_row(g("g1_as").reshape(-1)).astype(BF),
        a_d=rep_row(g("g1_ad").reshape(-1)).astype(BF),
        gb=rep_row(g("g1_b")), fb=g("fc1_b").reshape(P, 1).astype(np.float32),
    )]
    for i in range(2):
        layers.append(dict(
            gw=blocks(g("mg_W")[i]), fw=blocks(g("mfc_W")[i]),
            a_s=rep_row(g("mg_as")[i].reshape(-1)).astype(BF),
            a_d=rep_row(g("mg_ad")[i].reshape(-1)).astype(BF),
            gb=rep_row(g("mg_b")[i]), fb=g("mfc_b")[i].reshape(P, 1).astype(np.float32),
        ))
    layers.append(dict(
        gw=blocks(g("fg_W")), fw=blocks(g("ffc_W")),
        a_s=rep_row(g("fg_as").reshape(-1)).astype(BF),
        a_d=rep_row(g("fg_ad").reshape(-1)).astype(BF),
        gb=rep_row(g("fg_b")), fb=rep_row(g("ffc_b")),
    ))
    return layers


# --------------------------------------------------------------------------
# device program
# --------------------------------------------------------------------------
def build_program(st, n_cores):
    SH, W, TT, TAB, CHSZ = st["SH"], st["W"], st["TT"], st["TAB"], st["CHSZ"]
    PB = st["PB"]
    SHC = SH // NCHUNK
    plan = st["plan"]
    dt = mybir.dt
    f32, bf16, i16 = dt.float32, dt.bfloat16, dt.int16
    HL = [HEADS, HEADS, HEADS, 1]
    GNTMAX = max(g["ntiles"] for g in plan)
    SECMAX = max(s["ntiles"] for g in plan for s in g["secs"])
    SECPMAX = max(s["npairs"] for g in plan for s in g["secs"])

    nc = bacc.Bacc(None)

    def inp(name, shape, d):
        return nc.declare_dram_parameter(name, list(shape), d, isOutput=False)

    x_in = inp("x", (SH, P), f32)
    idx_in = inp("idx", (P, TT * 8), i16)
    s0_in = inp("s0", (P, PB * P), bf16)
    s0t_in = inp("s0t", (P, PB * P), bf16)
    iota_in = inp("iota8", (P, P), dt.int8)
    iotac_in = inp("iotac8", (P, 1), dt.int8)
    lw = []
    for L in range(4):
        K = 1 if L == 0 else 2
        lw.append(dict(
            gw=inp(f"gw{L}", (K, P, P), bf16),
            fw=inp(f"fw{L}", (K, P, P), bf16),
            a_s=inp(f"as{L}", (P, P), bf16),
            a_d=inp(f"ad{L}", (P, P), bf16),
            gb=inp(f"gb{L}", (P, P), f32),
            fb=inp(f"fb{L}", (P, 1) if L < 3 else (P, P), f32),
        ))
    out_t = nc.declare_dram_parameter("out", [SH, P], f32, isOutput=True)

    CH = 512
    chunks = [(c, min(CH, SH - c)) for c in range(0, SH, CH)]

    with tile.TileContext(nc) as tc:
        with (
            tc.tile_pool(name="res", bufs=1) as res,
            tc.tile_pool(name="wts", bufs=1) as wts,
            tc.tile_pool(name="nwork", bufs=3) as nwork,
            tc.tile_pool(name="ework", bufs=2) as ework,
            tc.tile_pool(name="psA", bufs=2, space="PSUM") as psA,
            tc.tile_pool(name="psB", bufs=2, space="PSUM") as psB,
            tc.tile_pool(name="dram", bufs=1, space="DRAM") as dram,
        ):
            # ---------------- residents
            iota8 = res.tile([P, P], dt.int8)
            nc.sync.dma_start(iota8[:], iota_in[:])
            iotac8 = res.tile([P, 1], dt.int8)
            nc.sync.dma_start(iotac8[:], iotac_in[:])
            ident = res.tile([P, P], bf16)
            nc.vector.tensor_tensor(
                out=ident[:], in0=iotac8[:].to_broadcast([P, P]), in1=iota8[:],
                op=mybir.AluOpType.is_equal)

            wt = []
            for L in range(4):
                K = 1 if L == 0 else 2
                d = {}
                for nm in ("gw", "fw"):
                    t_ = wts.tile([P, K, P], bf16, tag=f"{nm}{L}")
                    nc.sync.dma_start(t_[:], lw[L][nm][:].rearrange("k p q -> p k q"))
                    d[nm] = t_
                for nm in ("a_s", "a_d"):
                    t_ = wts.tile([P, P], bf16, tag=f"{nm}{L}")
                    nc.sync.dma_start(t_[:], lw[L][nm][:])
                    d[nm] = t_
                t_ = wts.tile([P, P], f32, tag=f"gb{L}")
                nc.sync.dma_start(t_[:], lw[L]["gb"][:])
                d["gb"] = t_
                t_ = wts.tile([P, 1] if L < 3 else [P, P], f32, tag=f"fb{L}")
                nc.sync.dma_start(t_[:], lw[L]["fb"][:])
                d["fb"] = t_
                wt.append(d)

            # ---------------- DRAM scratch
            sA = [dram.tile([P, SH], bf16, tag=f"sA{i}", name=f"sA{i}") for i in range(3)]
            sB = [dram.tile([P, SH], bf16, tag=f"sB{i}", name=f"sB{i}") for i in range(3)]
            h_bounce = dram.tile([SH, P], bf16, tag="hb")
            tables = [[dram.tile([CHSZ, P], bf16, tag=f"tab{L}_{c}",
                                 name=f"tab{L}_{c}", addr_space="Shared")
                       for c in range(NCHUNK)] for L in range(4)]
            x4_dram = dram.tile([SH, P], bf16, tag="x4")

            # ---------------- x -> transposed state
            for i in range(W):
                xt = nwork.tile([P, P], f32, tag="xin")
                nc.sync.dma_start(xt[:], x_in[i * P:(i + 1) * P, :])
                xb = nwork.tile([P, P], bf16, tag="xbf")
                nc.vector.tensor_copy(out=xb[:], in_=xt[:])
                tp = psB.tile([P, P], bf16, tag="tp", bufs=1)
                nc.tensor.transpose(out=tp[:], in_=xb[:], identity=ident[:])
                xTb = nwork.tile([P, P], bf16, tag="xT")
                nc.vector.tensor_copy(out=xTb[:], in_=tp[:])
                nc.sync.dma_start(sA[0][:, i * P:(i + 1) * P], xTb[:])

            # ---------------- layers
            for L in range(4):
                K = 1 if L == 0 else 2
                H = HL[L]
                C = P // H
                w = wt[L]
                resid = False
                if L == 0:
                    in_blk = [sA[0]]
                elif L == 1:
                    in_blk = [sA[1], sB[1]]
                elif L == 2:
                    resid = True
                    in_blk = [(sA[1], sA[2]), (sB[1], sB[2])]
                else:
                    in_blk = [sA[1], sB[1]]

                # ---- node phase (fused h + x1 + ald, AG interleaved)
                ald_sb = res.tile([P, W, 4], f32, tag=f"aldsb{L % 2}")
                if H < 4:
                    nc.vector.memset(ald_sb[:], 0.0)
                outA = [sA[1], sA[2], sA[1], None][L]
                next_ag = 0
                for c0, cl in chunks:
                    nwin = cl // P
                    ins = []
                    for k in range(K):
                        it = nwork.tile([P, CH], bf16, tag=f"in{k}")
                        if resid:
                            b1, b2 = in_blk[k]
                            nc.sync.dma_start(it[:, :cl], b1[:, c0:c0 + cl])
                            it2 = nwork.tile([P, CH], bf16, tag=f"in2{k}")
                            nc.sync.dma_start(it2[:, :cl], b2[:, c0:c0 + cl])
                            nc.vector.tensor_add(out=it[:, :cl], in0=it[:, :cl],
                                                 in1=it2[:, :cl])
                        else:
                            nc.sync.dma_start(it[:, :cl], in_blk[k][:, c0:c0 + cl])
                        ins.append(it)
                    # x1 branch
                    if L < 3:
                        acc = psB.tile([P, CH], f32, tag="x1p")
                        for k in range(K):
                            nc.tensor.matmul(out=acc[:, :cl], lhsT=w["fw"][:, k, :],
                                             rhs=ins[k][:, :cl],
                                             start=(k == 0), stop=(k == K - 1))
                        x1b = nwork.tile([P, CH], bf16, tag="x1b")
                        nc.scalar.activation(out=x1b[:, :cl], in_=acc[:, :cl],
                                             func=mybir.ActivationFunctionType.Relu,
                                             bias=w["fb"][:], scale=1.0)
                        nc.sync.dma_start(outA[:, c0:c0 + cl], x1b[:, :cl])
                    # gat h per window (+ x4 for L==3)
                    for j in range(nwin):
                        i = c0 // P + j
                        sl = slice(j * P, (j + 1) * P)
                        hp = psA.tile([P, P], f32, tag="hp")
                        for k in range(K):
                            nc.tensor.matmul(out=hp[:], lhsT=ins[k][:, sl],
                                             rhs=w["gw"][:, k, :],
                                             start=(k == 0), stop=(k == K - 1))
                        hb = nwork.tile([P, P], bf16, tag="hbf")
                        nc.vector.tensor_copy(out=hb[:], in_=hp[:])
                        nc.sync.dma_start(h_bounce[i * P:(i + 1) * P, :], hb[:])
                        tm = nwork.tile([P, P], bf16, tag="adtmp")
                        nc.vector.tensor_tensor(out=tm[:], in0=hb[:], in1=w["a_d"][:],
                                                op=mybir.AluOpType.mult)
                        nc.vector.reduce_sum(
                            out=ald_sb[:, i, 0:H],
                            in_=tm[:].rearrange("p (h c) -> p h c", h=H),
                            axis=mybir.AxisListType.X)
                        if L == 3:
                            xp = psB.tile([P, CH], f32, tag="x1p")
                            for k in range(K):
                                nc.tensor.matmul(out=xp[:, :P], lhsT=ins[k][:, sl],
                                                 rhs=w["fw"][:, k, :],
                                                 start=(k == 0), stop=(k == K - 1))
                            x4t = nwork.tile([P, P], f32, tag="x4t")
                            nc.vector.tensor_add(out=x4t[:], in0=xp[:, :P], in1=w["fb"][:])
                            nc.vector.tensor_scalar_max(out=x4t[:], in0=x4t[:], scalar1=0.0)
                            x4b = nwork.tile([P, P], bf16, tag="x4b")
                            nc.vector.tensor_copy(out=x4b[:], in_=x4t[:])
                            nc.sync.dma_start(x4_dram[i * P:(i + 1) * P, :], x4b[:])
                        # fire chunk AG ops whose input rows are now complete
                        while next_ag < NCHUNK and (next_ag + 1) * SHC <= (i + 1) * P:
                            a0 = next_ag * SHC
                            nc.gpsimd.collective_compute(
                                "AllGather", mybir.AluOpType.bypass,
                                replica_groups=[list(range(n_cores))],
                                ins=[h_bounce[a0:a0 + SHC, :]],
                                outs=[tables[L][next_ag][:]],
                            )
                            next_ag += 1
                ald_bf = res.tile([P, W * 4], bf16, tag=f"aldbf{L % 2}")
                nc.vector.tensor_copy(out=ald_bf[:], in_=ald_sb[:].rearrange("p w c -> p (w c)"))

                # ---- edge phase
                for g in plan:
                    gt0, gnt = g["off"], g["ntiles"]
                    idxg = ework.tile([P, GNTMAX * 8], i16, tag="idxg")
                    nc.sync.dma_start(idxg[:, :gnt * 8], idx_in[:, gt0 * 8:(gt0 + gnt) * 8])
                    G = ework.tile([P, GNTMAX, P], bf16, tag="G")
                    for c in range(NCHUNK):
                        sec = g["secs"][c]
                        nt = sec["ntiles"]
                        if nt == 0:
                            continue
                        sl = sec["off"] - gt0
                        nc.gpsimd.dma_gather(
                            G[:, sl:sl + nt, :],
                            tables[L][c][:],
                            idxg[:, sl * 8:(sl + nt) * 8],
                            num_idxs=nt * P, num_idxs_reg=nt * P, elem_size=P,
                            single_packet=False)

                    als = ework.tile([P, GNTMAX * 4], f32, tag="als")
                    lg = ework.tile([P, GNTMAX * 4], f32, tag="lg")
                    for c in range(NCHUNK):
                        sec = g["secs"][c]
                        nt = sec["ntiles"]
                        if nt == 0:
                            continue
                        sl = sec["off"] - gt0
                        # --- attention logits: als (fold then reduce)
                        tmp = ework.tile([P, SECMAX, P], bf16, tag="tmp")
                        nc.vector.tensor_tensor(
                            out=tmp[:, :nt, :], in0=G[:, sl:sl + nt, :],
                            in1=w["a_s"][:].rearrange("p q -> p () q").to_broadcast([P, nt, P]),
                            op=mybir.AluOpType.mult)
                        tmpv = tmp[:, :nt, :].rearrange("p t (h c) -> p (t h) c", h=H)
                        nc.vector.tensor_add(
                            out=tmpv[:, :, 0:C // 2], in0=tmpv[:, :, 0:C // 2],
                            in1=tmpv[:, :, C // 2:C])
                        nc.vector.tensor_add(
                            out=tmpv[:, :, 0:C // 4], in0=tmpv[:, :, 0:C // 4],
                            in1=tmpv[:, :, C // 4:C // 2])
                        nc.vector.reduce_sum(
                            out=als[:, sl * H:(sl + nt) * H],
                            in_=tmpv[:, :, 0:C // 4],
                            axis=mybir.AxisListType.X)
                        # --- per-edge ald via host S0T one-hot matmul
                        npr = sec["npairs"]
                        pbase = sec["pbase"]
                        s0t_sec = ework.tile([P, SECPMAX * P], bf16, tag="s0t")
                        nc.sync.dma_start(
                            s0t_sec[:, :npr * P],
                            s0t_in[:, pbase * P:(pbase + npr) * P])
                        aldp = psA.tile([P, SECMAX, 4], f32, tag="ald", bufs=1)
                        for lt in range(nt):
                            prs = sec["tile_pairs"][lt]
                            for j, (wj, lp) in enumerate(prs):
                                nc.tensor.matmul(
                                    out=aldp[:, lt, 0:H],
                                    lhsT=s0t_sec[:, lp * P:(lp + 1) * P],
                                    rhs=ald_bf[:, wj * 4:wj * 4 + H],
                                    start=(j == 0), stop=(j == len(prs) - 1))
                        nc.vector.tensor_add(
                            out=lg[:, sl * H:(sl + nt) * H],
                            in0=als[:, sl * H:(sl + nt) * H],
                            in1=aldp[:, :nt, 0:H].rearrange("p t h -> p (t h)"))

                    lr = ework.tile([P, GNTMAX * 4], f32, tag="lr")
                    nc.vector.scalar_tensor_tensor(
                        out=lr[:, :gnt * H], in0=lg[:, :gnt * H], scalar=0.2,
                        in1=lg[:, :gnt * H],
                        op0=mybir.AluOpType.mult, op1=mybir.AluOpType.max)

                    x2acc = ework.tile([P, GB, P + 4], f32, tag="x2acc")
                    wdone = {}
                    for c in range(NCHUNK):
                        sec = g["secs"][c]
                        nt = sec["ntiles"]
                        if nt == 0:
                            continue
                        sl = sec["off"] - gt0
                        # pe expanded to [P, nt, H, C] on the scalar engine
                        pex = ework.tile([P, SECMAX, P], bf16, tag="pex")
                        nc.scalar.activation(
                            out=pex[:, :nt, :].rearrange("p t (h c) -> p t h c", h=H),
                            in_=lr[:, sl * H:(sl + nt) * H]
                                .rearrange("p (t h) -> p t h ()", h=H)
                                .to_broadcast([P, nt, H, C]),
                            func=mybir.ActivationFunctionType.Exp)
                        pe_t = ework.tile([P, SECMAX * 4], f32, tag="pe")
                        nc.scalar.activation(
                            out=pe_t[:, :nt * H], in_=lr[:, sl * H:(sl + nt) * H],
                            func=mybir.ActivationFunctionType.Exp)
                        GW = ework.tile([P, SECMAX, P + 4], bf16, tag="GW")
                        nc.vector.tensor_tensor(
                            out=GW[:, :nt, 0:P],
                            in0=G[:, sl:sl + nt, :],
                            in1=pex[:, :nt, :],
                            op=mybir.AluOpType.mult)
                        nc.vector.tensor_copy(
                            out=GW[:, :nt, P:P + H],
                            in_=pe_t[:, :nt * H].rearrange("p (t h) -> p t h", h=H))
                        s0_sec = ework.tile([P, SECPMAX * P], bf16, tag="s0")
                        nc.sync.dma_start(
                            s0_sec[:, :sec["npairs"] * P],
                            s0_in[:, sec["pbase"] * P:(sec["pbase"] + sec["npairs"]) * P])
                        for (wi, run) in sec["win_runs"]:
                            aggp = psA.tile([P, P + 4], f32, tag="agg")
                            for j, (lt, lp) in enumerate(run):
                                nc.tensor.matmul(
                                    out=aggp[:, :P + H],
                                    lhsT=s0_sec[:, lp * P:(lp + 1) * P],
                                    rhs=GW[:, lt, 0:P + H],
                                    start=(j == 0), stop=(j == len(run) - 1))
                            wl = wi - g["windows"][0]
                            if wi not in wdone:
                                wdone[wi] = True
                                nc.vector.tensor_copy(
                                    out=x2acc[:, wl, 0:P + H], in_=aggp[:, :P + H])
                            else:
                                nc.vector.tensor_add(
                                    out=x2acc[:, wl, 0:P + H],
                                    in0=x2acc[:, wl, 0:P + H], in1=aggp[:, :P + H])

                    for wi in g["windows"]:
                        wl = wi - g["windows"][0]
                        if wi not in wdone:
                            continue
                        sinv = ework.tile([P, 4], f32, tag="sinv")
                        nc.vector.tensor_scalar_add(
                            out=sinv[:, :H], in0=x2acc[:, wl, P:P + H], scalar1=1e-16)
                        nc.vector.reciprocal(out=sinv[:, :H], in_=sinv[:, :H])
                        x2 = ework.tile([P, P], f32, tag="x2")
                        nc.vector.tensor_tensor(
                            out=x2[:].rearrange("p (h c) -> p h c", h=H),
                            in0=x2acc[:, wl, 0:P].rearrange("p (h c) -> p h c", h=H),
                            in1=sinv[:, :H].rearrange("p h -> p h ()").to_broadcast([P, H, C]),
                            op=mybir.AluOpType.mult)
                        nc.vector.tensor_add(out=x2[:], in0=x2[:], in1=w["gb"][:])
                        nc.vector.tensor_scalar_max(out=x2[:], in0=x2[:], scalar1=0.0)
                        if L < 3:
                            x2b = ework.tile([P, P], bf16, tag="x2b")
                            nc.vector.tensor_copy(out=x2b[:], in_=x2[:])
                            tp = psB.tile([P, P], bf16, tag="tp", bufs=1)
                            nc.tensor.transpose(out=tp[:], in_=x2b[:], identity=ident[:])
                            x2T = ework.tile([P, P], bf16, tag="x2T")
                            nc.vector.tensor_copy(out=x2T[:], in_=tp[:])
                            outB = [sB[1], sB[2], sB[1]][L]
                            nc.sync.dma_start(outB[:, wi * P:(wi + 1) * P], x2T[:])
                        else:
                            x4t = ework.tile([P, P], bf16, tag="x4in")
                            nc.sync.dma_start(x4t[:], x4_dram[wi * P:(wi + 1) * P, :])
                            yo = ework.tile([P, P], f32, tag="yo")
                            nc.vector.tensor_add(out=yo[:], in0=x2[:], in1=x4t[:])
                            nc.sync.dma_start(out_t[wi * P:(wi + 1) * P, :], yo[:])

    nc.compile()
    return nc


# --------------------------------------------------------------------------
# runner
# --------------------------------------------------------------------------
def make_in_maps(inputs, st):
    x = np.asarray(inputs["x"], np.float32)
    shard, SH = st["shard"], st["SH"]
    layers = prep_weights(inputs)
    iota8 = np.broadcast_to(np.arange(P, dtype=np.int8), (P, P)).copy()
    iotac8 = np.arange(P, dtype=np.int8).reshape(P, 1).copy()

    common = {"iota8": iota8, "iotac8": iotac8}
    for L, lwd in enumerate(layers):
        common[f"gw{L}"] = lwd["gw"]
        common[f"fw{L}"] = lwd["fw"]
        common[f"as{L}"] = lwd["a_s"]
        common[f"ad{L}"] = lwd["a_d"]
        common[f"gb{L}"] = lwd["gb"]
        common[f"fb{L}"] = lwd["fb"]

    in_maps = []
    for c in range(N_CORES):
        xs = np.zeros((SH, P), np.float32)
        xs[:shard] = x[c * shard:(c + 1) * shard]
        m = dict(common)
        m["x"] = xs
        m["idx"] = st["idx"][c]
        m["s0"] = st["s0"][c]
        m["s0t"] = st["s0t"][c]
        in_maps.append(m)
    return in_maps


_CACHE = {}


def run(inputs, trace=False):
    from concourse.bass_utils import run_bass_kernel_spmd

    st = prep_structure(np.asarray(inputs["edge_index"]), N_NODES, N_CORES)
    key = (st["SH"], st["TT"])
    if key not in _CACHE:
        _CACHE[key] = build_program(st, N_CORES)
    nc = _CACHE[key]
    in_maps = make_in_maps(inputs, st)
    res = run_bass_kernel_spmd(nc, in_maps, core_ids=list(range(N_CORES)),
                               trace=trace)
    outs = [np.asarray(res.results[c]["out"])[:st["shard"]] for c in range(N_CORES)]
    return np.concatenate(outs, axis=0).astype(np.float32), res


def kernel(**inputs):
    out, _ = run(inputs, trace=False)
    return out
